# revision 8
# baseline (speedup 1.0000x reference)
"""MultiHeadCrossAttentionWithMask on 8 Trainium2 NeuronCores.

Sharding: core i handles batch b = i // 4 and head group hg = i % 4
(4 heads of 16). Data-parallel over B, tensor-parallel over heads for the
QKV projections; the output projection is computed as per-core partial
sums over the head shard and reduced (plus bias) on the host.

Per-core device program (all matmuls in float32r, full PE rate):
  1. LayerNorm of q/k/v inputs in natural [s,D] layout (bn_stats),
     PE-transpose of the normalized tiles to [D,s].
  2. QKV projections with head-sharded weights; Q^T/K^T stored per head
     as [65, S] tiles: row 64 of Q^T = ones, row 64 of K^T = additive
     mask (-1e9 at masked keys) so scores = Q.K^T + mask comes out of a
     single K=65 matmul. V^T is PE-transposed back to [s, hd] layout.
  3. S-path (probs): per q-tile, scores -> exp (free-dim sum via
     accum_out) -> scale by 1/Z -> DMA probs rows to HBM.
  4. S^T-path: scores transposed [k, q] -> exp with per-partition mask
     bias -> PV matmuls accumulate unnormalized ctx^T; scaled by 1/Z
     (broadcast via a DRAM bounce) into f32r tiles.
  5. Output projection over the 4 local heads -> partial context.
"""

import numpy as np

import concourse.bass as bass
import concourse.tile as tile
from concourse import bacc, mybir
from concourse.masks import make_identity

B, S, D, H, HD = 2, 2048, 1024, 16, 64
NH = 4  # heads per core
DM = NH * HD  # 256 projection cols per core
EPS = 1e-5
NEG = -1.0e9

f32 = mybir.dt.float32
f32r = mybir.dt.float32r
AF = mybir.ActivationFunctionType
ALU = mybir.AluOpType

_CACHED_NC = None
_CACHED_RUNNER = None
_ONES = np.ones((S,), np.float32)


class _Runner:
    """Cached PJRT executor for the compiled Bass module on 8 cores.

    Mirrors concourse.bass2jax.run_bass_via_pjrt's multi-core path, but
    caches the jitted shard_map executable and creates the donated zero
    output buffers on device (no 576MB host->device zero upload per call).
    """

    def __init__(self, nc, n_cores=8):
        import jax
        import jax.numpy as jnp
        from jax.experimental.shard_map import shard_map
        from jax.sharding import Mesh, NamedSharding, PartitionSpec
        from concourse.bass2jax import (
            _bass_exec_p,
            install_neuronx_cc_hook,
            partition_id_tensor,
        )

        install_neuronx_cc_hook()
        self.jax = jax
        self.np = np
        self.n_cores = n_cores

        in_names, out_names, out_avals = [], [], []
        for alloc in nc.m.functions[0].allocations:
            if not isinstance(alloc, mybir.MemoryLocationSet):
                continue
            name = alloc.memorylocations[0].name
            if alloc.kind == "ExternalInput":
                if (
                    nc.partition_id_tensor is None
                    or name != nc.partition_id_tensor.name
                ):
                    in_names.append(name)
            elif alloc.kind == "ExternalOutput":
                out_names.append(name)
                shape = tuple(alloc.tensor_shape)
                dtype = mybir.dt.np(alloc.dtype)
                out_avals.append(jax.core.ShapedArray(shape, dtype))
        self.in_names = in_names
        self.out_names = out_names
        self.out_avals = out_avals
        n_params = len(in_names)
        n_outs = len(out_avals)
        all_in_names = list(in_names) + list(out_names)
        partition_name = (
            nc.partition_id_tensor.name if nc.partition_id_tensor else None
        )
        if partition_name is not None:
            all_in_names.append(partition_name)

        devices = jax.devices()[:n_cores]
        assert len(devices) >= n_cores
        self.mesh = Mesh(np.asarray(devices[:n_cores]), ("core",))
        self.sharding = NamedSharding(self.mesh, PartitionSpec("core"))

        def _body(*args):
            operands = list(args)
            if partition_name is not None:
                operands.append(partition_id_tensor())
            outs = _bass_exec_p.bind(
                *operands,
                out_avals=tuple(out_avals),
                in_names=tuple(all_in_names),
                out_names=tuple(out_names),
                lowering_input_output_aliases=(),
                sim_require_finite=True,
                sim_require_nnan=True,
                nc=nc,
            )
            return tuple(outs)

        donate = tuple(range(n_params, n_params + n_outs))
        self.sharded = jax.jit(
            shard_map(
                _body,
                mesh=self.mesh,
                in_specs=(PartitionSpec("core"),) * (n_params + n_outs),
                out_specs=(PartitionSpec("core"),) * n_outs,
                check_rep=False,
            ),
            donate_argnums=donate,
            keep_unused=True,
        )

        zero_shapes = [(n_cores * a.shape[0], *a.shape[1:]) for a in out_avals]
        zero_dtypes = [a.dtype for a in out_avals]
        self._make_zeros = jax.jit(
            lambda: tuple(
                jnp.zeros(s, d) for s, d in zip(zero_shapes, zero_dtypes)
            ),
            out_shardings=(self.sharding,) * n_outs,
        )

    def stage_inputs(self, in_maps):
        staged = []
        for name in self.in_names:
            cat = np.concatenate([np.asarray(m[name]) for m in in_maps], axis=0)
            staged.append(self.jax.device_put(cat, self.sharding))
        self.jax.block_until_ready(staged)
        return staged

    def execute(self, staged_inputs):
        zeros = self._make_zeros()
        self.jax.block_until_ready(zeros)
        outs = self.sharded(*staged_inputs, *zeros)
        self.jax.block_until_ready(outs)
        return outs

    def to_numpy(self, outs):
        res = []
        big = [np.asarray(o) for o in outs]
        for c in range(self.n_cores):
            d = {}
            for i, name in enumerate(self.out_names):
                d[name] = big[i].reshape(
                    self.n_cores, *self.out_avals[i].shape
                )[c]
            res.append(d)
        return res


def _build():
    nc = bacc.Bacc("TRN2", target_bir_lowering=False, debug=False, num_devices=8)

    xq = nc.dram_tensor("xq", [S, D], f32, kind="ExternalInput").ap()
    xk = nc.dram_tensor("xk", [S, D], f32, kind="ExternalInput").ap()
    xv = nc.dram_tensor("xv", [S, D], f32, kind="ExternalInput").ap()
    wqT = nc.dram_tensor("wqT", [D, DM], f32, kind="ExternalInput").ap()
    wkT = nc.dram_tensor("wkT", [D, DM], f32, kind="ExternalInput").ap()
    wvT = nc.dram_tensor("wvT", [D, DM], f32, kind="ExternalInput").ap()
    woT = nc.dram_tensor("woT", [DM, D], f32, kind="ExternalInput").ap()
    bq = nc.dram_tensor("bq", [DM], f32, kind="ExternalInput").ap()
    bk = nc.dram_tensor("bk", [DM], f32, kind="ExternalInput").ap()
    bv = nc.dram_tensor("bv", [DM], f32, kind="ExternalInput").ap()
    amask = nc.dram_tensor("amask", [S], f32, kind="ExternalInput").ap()
    ones_d = nc.dram_tensor("ones_d", [S], f32, kind="ExternalInput").ap()

    probs4 = nc.dram_tensor("probs4", [NH, S, S], f32, kind="ExternalOutput").ap()
    ctxp = nc.dram_tensor("ctxp", [S, D], f32, kind="ExternalOutput").ap()

    with tile.TileContext(nc) as tc:
        with (
            tc.tile_pool(name="persist", bufs=1) as pp,
            tc.tile_pool(name="dram", bufs=1, space="DRAM") as dp,
            tc.tile_pool(name="ps_a", bufs=2, space="PSUM") as ps_a,  # tr/proj/oproj
            tc.tile_pool(name="ps_s", bufs=1, space="PSUM") as ps_sp,  # [128,1024]
            tc.tile_pool(name="ps_st", bufs=1, space="PSUM") as ps_st,
            tc.tile_pool(name="ps_ctx", bufs=1, space="PSUM") as ps_ctx,
        ):
            # ---------- persistent tiles ----------
            b2 = [pp.tile([128, 2], f32, name=f"b2_{t}") for t in range(3)]
            for t, bdram in enumerate([bq, bk, bv]):
                nc.sync.dma_start(out=b2[t], in_=bdram.rearrange("(t p) -> p t", p=128))

            amaskT = pp.tile([128, 16], f32)
            nc.sync.dma_start(out=amaskT, in_=amask.rearrange("(t p) -> p t", p=128))

            eps_t = pp.tile([128, 1], f32)
            nc.vector.memset(eps_t, EPS)

            # per-head Q^T/K^T with extra fused row (ones / additive mask)
            qT65 = [pp.tile([65, S], f32r, name=f"qT65_{h}") for h in range(NH)]
            kT65 = [pp.tile([65, S], f32r, name=f"kT65_{h}") for h in range(NH)]
            for h in range(NH):
                nc.gpsimd.dma_start(out=qT65[h][64:65, :], in_=ones_d[None, :])
                nc.gpsimd.dma_start(out=kT65[h][64:65, :], in_=amask[None, :])

            # V in [k, d] layout: per head-pair, free dim = (kt, dm-pair 128)
            vvp = [pp.tile([128, S], f32r, name=f"vvp{hp}") for hp in range(2)]
            # scaled ctx^T per head
            ctxTs = [pp.tile([64, S], f32r, name=f"ctxTs{h}") for h in range(NH)]
            # 1/Z per (head, q-tile): [128, 16]
            zinv_qt = [pp.tile([128, 16], f32, name=f"zinv{h}") for h in range(NH)]
            zd = [dp.tile([S], f32, name=f"zd{h}") for h in range(NH)]

            # ---------- phase 1: LN + transpose + projections ----------
            with (
                tc.tile_pool(name="wp", bufs=1) as wp,
                tc.tile_pool(name="ln", bufs=3) as lnp,
                tc.tile_pool(name="xnp", bufs=4) as xnp,
                tc.tile_pool(name="xtp", bufs=9) as xtp,
                tc.tile_pool(name="stage", bufs=2) as stp,
            ):
                ident = wp.tile([128, 128], f32)
                make_identity(nc, ident)
                ident_r = wp.tile([128, 128], f32r)
                nc.gpsimd.dma_start(out=ident_r, in_=ident)
                wdram = [wqT, wkT, wvT]
                wt = [
                    [
                        [
                            wp.tile([128, 128], f32r, name=f"w{t}_{dt}_{mt}")
                            for mt in range(2)
                        ]
                        for dt in range(8)
                    ]
                    for t in range(3)
                ]
                for t in range(3):
                    for dt in range(8):
                        for mt in range(2):
                            nc.gpsimd.dma_start(
                                out=wt[t][dt][mt],
                                in_=wdram[t][
                                    dt * 128 : (dt + 1) * 128,
                                    mt * 128 : (mt + 1) * 128,
                                ],
                            )
                for sb in range(4):
                    for t in range(3):
                        xdram = [xq, xk, xv][t]
                        xns = []
                        for st4 in range(4):
                            st = sb * 4 + st4
                            x_t = lnp.tile([128, D], f32, tag="x")
                            nc.sync.dma_start(
                                out=x_t, in_=xdram[st * 128 : (st + 1) * 128, :]
                            )
                            stats = lnp.tile([128, 2, 6], f32, tag="stats")
                            xg = x_t.rearrange("p (g d) -> p g d", g=2)
                            for g in range(2):
                                nc.vector.bn_stats(
                                    out=stats[:, g, :], in_=xg[:, g, :]
                                )
                            mv = lnp.tile([128, 2], f32, tag="mv")
                            nc.vector.bn_aggr(out=mv, in_=stats)
                            rstd = lnp.tile([128, 1], f32, tag="rstd")
                            nc.scalar.activation(
                                out=rstd,
                                in_=mv[:, 1:2],
                                func=AF.Sqrt,
                                bias=eps_t,
                                scale=1.0,
                            )
                            nc.vector.reciprocal(out=rstd, in_=rstd)
                            xn = xnp.tile([128, D], f32, tag="xn")
                            nc.vector.tensor_scalar(
                                out=xn,
                                in0=x_t,
                                scalar1=mv[:, 0:1],
                                scalar2=rstd,
                                op0=ALU.subtract,
                                op1=ALU.mult,
                            )
                            xns.append(xn)
                        # transpose to [D, s] for this s-block
                        xT = []
                        for dt in range(8):
                            ptr = ps_a.tile([128, 512], f32, tag="psa")
                            for st4 in range(4):
                                nc.tensor.transpose(
                                    ptr[:, st4 * 128 : (st4 + 1) * 128],
                                    xns[st4][:, dt * 128 : (dt + 1) * 128],
                                    ident,
                                )
                            xb = xtp.tile([128, 512], f32r, tag="xt")
                            nc.vector.tensor_copy(out=xb, in_=ptr)
                            xT.append(xb)
                        # projection for this tensor and s-block
                        for mt in range(2):
                            pj = ps_a.tile([128, 512], f32, tag="psa")
                            for dt in range(8):
                                nc.tensor.matmul(
                                    pj[:],
                                    lhsT=wt[t][dt][mt],
                                    rhs=xT[dt][:],
                                    start=(dt == 0),
                                    stop=(dt == 7),
                                )
                            stg = stp.tile([128, 512], f32r, tag=f"stg{t % 2}")
                            nc.vector.tensor_scalar_add(
                                out=stg, in0=pj, scalar1=b2[t][:, mt : mt + 1]
                            )
                            sl = slice(sb * 512, (sb + 1) * 512)
                            if t == 0 or t == 1:
                                dst = qT65 if t == 0 else kT65
                                nc.sync.dma_start(
                                    out=dst[2 * mt][0:64, sl], in_=stg[0:64, :]
                                )
                                nc.sync.dma_start(
                                    out=dst[2 * mt + 1][0:64, sl], in_=stg[64:128, :]
                                )
                            else:
                                # V: transpose [dm-pair, s] -> [s, dm-pair]
                                ptv = ps_a.tile([128, 512], f32r, tag="psa")
                                for st4 in range(4):
                                    nc.tensor.transpose(
                                        ptv[:, st4 * 128 : (st4 + 1) * 128],
                                        stg[:, st4 * 128 : (st4 + 1) * 128],
                                        ident_r,
                                    )
                                nc.vector.tensor_copy(out=vvp[mt][:, sl], in_=ptv)

            # ---------- phase 2: attention ----------
            with (
                tc.tile_pool(name="probs_t", bufs=3) as ptp,
                tc.tile_pool(name="et", bufs=2) as etp,
                tc.tile_pool(name="zt", bufs=4) as ztp,
                tc.tile_pool(name="osb", bufs=3) as osp,
                tc.tile_pool(name="wop", bufs=1) as wop,
                tc.tile_pool(name="zbp", bufs=1) as zbp,
            ):
                woTh = [wop.tile([64, D], f32r, name=f"woTh{h}") for h in range(NH)]
                for h in range(NH):
                    nc.gpsimd.dma_start(
                        out=woTh[h], in_=woT[h * 64 : (h + 1) * 64, :]
                    )
                zbcast = {}
                for hp in range(2):
                    for hi in range(2):
                        h = 2 * hp + hi
                        # ---- S path: probs + Z ----
                        for qt in range(16):
                            ptile = ptp.tile([128, S], f32, tag="ptile")
                            zpart = ztp.tile([128, 2], f32, tag="zpart")
                            for kb in range(2):
                                pss = ps_sp.tile([128, 1024], f32, tag="pss")
                                for kk in range(2):
                                    nc.tensor.matmul(
                                        pss[:, kk * 512 : (kk + 1) * 512],
                                        lhsT=qT65[h][:, qt * 128 : (qt + 1) * 128],
                                        rhs=kT65[h][
                                            :,
                                            kb * 1024
                                            + kk * 512 : kb * 1024
                                            + (kk + 1) * 512,
                                        ],
                                        start=True,
                                        stop=True,
                                    )
                                nc.scalar.activation(
                                    out=ptile[:, kb * 1024 : (kb + 1) * 1024],
                                    in_=pss[:],
                                    func=AF.Exp,
                                    accum_out=zpart[:, kb : kb + 1],
                                )
                            zs = ztp.tile([128, 1], f32, tag="zs")
                            nc.vector.reduce_sum(
                                out=zs, in_=zpart, axis=mybir.AxisListType.X
                            )
                            nc.vector.reciprocal(
                                out=zinv_qt[h][:, qt : qt + 1], in_=zs
                            )
                            nc.vector.tensor_scalar_mul(
                                out=ptile,
                                in0=ptile,
                                scalar1=zinv_qt[h][:, qt : qt + 1],
                            )
                            nc.sync.dma_start(
                                out=probs4[h, qt * 128 : (qt + 1) * 128, :],
                                in_=ptile[:],
                            )
                        # 1/Z into free-layout broadcast tile via DRAM bounce
                        nc.sync.dma_start(
                            out=zd[h].rearrange("(t p) -> p t", p=128),
                            in_=zinv_qt[h][:],
                        )
                        zbcast[h] = zbp.tile([64, S], f32, tag=f"zb{hi}", name=f"zbc{hi}")
                        nc.sync.dma_start(
                            out=zbcast[h][:],
                            in_=zd[h][None, :].broadcast_to([64, S]),
                        )

                    # ---- S^T + PV for the pair ----
                    for qb in range(4):
                        qsl = slice(qb * 512, (qb + 1) * 512)
                        pcA = ps_ctx.tile([64, 512], f32, tag="ctxA")
                        pcB = ps_ctx.tile([64, 512], f32, tag="ctxB")
                        for kt in range(16):
                            ksl = slice(kt * 128, (kt + 1) * 128)
                            for hi, pc in ((0, pcA), (1, pcB)):
                                h = 2 * hp + hi
                                pst = ps_st.tile(
                                    [128, 512], f32, tag=f"st{hi}"
                                )
                                nc.tensor.matmul(
                                    pst[:],
                                    lhsT=kT65[h][0:64, ksl],
                                    rhs=qT65[h][0:64, qsl],
                                    start=True,
                                    stop=True,
                                )
                                eT = etp.tile([128, 512], f32r, tag=f"eT{hi}")
                                nc.scalar.activation(
                                    out=eT,
                                    in_=pst,
                                    func=AF.Exp,
                                    bias=amaskT[:, kt : kt + 1],
                                )
                                nc.tensor.matmul(
                                    pc[:],
                                    lhsT=vvp[hp][
                                        :, kt * 128 + hi * 64 : kt * 128 + hi * 64 + 64
                                    ],
                                    rhs=eT[:],
                                    start=(kt == 0),
                                    stop=(kt == 15),
                                )
                        for hi, pc in ((0, pcA), (1, pcB)):
                            h = 2 * hp + hi
                            nc.vector.tensor_mul(
                                ctxTs[h][:, qsl], pc[:], zbcast[h][:, qsl]
                            )

                # ---------- phase 3: output projection ----------
                for qt in range(16):
                    osb = osp.tile([128, D], f32, tag="osb")
                    for nt in range(2):
                        po = ps_a.tile([128, 512], f32, tag="psa")
                        for h in range(NH):
                            nc.tensor.matmul(
                                po[:],
                                lhsT=ctxTs[h][:, qt * 128 : (qt + 1) * 128],
                                rhs=woTh[h][:, nt * 512 : (nt + 1) * 512],
                                start=(h == 0),
                                stop=(h == NH - 1),
                            )
                        nc.vector.tensor_copy(
                            out=osb[:, nt * 512 : (nt + 1) * 512], in_=po
                        )
                    nc.sync.dma_start(
                        out=ctxp[qt * 128 : (qt + 1) * 128, :], in_=osb[:]
                    )

    nc.compile()
    return nc


def _get_nc():
    global _CACHED_NC
    if _CACHED_NC is None:
        _CACHED_NC = _build()
    return _CACHED_NC


def _get_runner():
    global _CACHED_RUNNER
    if _CACHED_RUNNER is None:
        _CACHED_RUNNER = _Runner(_get_nc(), 8)
    return _CACHED_RUNNER


def kernel(query, key, value, mask, gamma, Wq, bq, Wk, bk, Wv, bv, Wo, bo, **kwargs):
    query = np.asarray(query, np.float32)
    key = np.asarray(key, np.float32)
    value = np.asarray(value, np.float32)
    mask = np.asarray(mask)
    gamma = np.asarray(gamma, np.float32)
    Wq, Wk, Wv, Wo = (np.asarray(w, np.float32) for w in (Wq, Wk, Wv, Wo))
    bq, bk, bv, bo = (np.asarray(b, np.float32) for b in (bq, bk, bv, bo))

    # fold gamma into the projection weights (q/k/v all consume LN(x)*gamma)
    Wqg = Wq * gamma[None, :]
    Wkg = Wk * gamma[None, :]
    Wvg = Wv * gamma[None, :]

    amask_np = ((mask != 0).astype(np.float32) - 1.0) * -NEG  # 0 or -1e9
    amask_np = np.ascontiguousarray(amask_np.astype(np.float32))

    in_maps = []
    for core in range(8):
        b = core // 4
        hg = core % 4
        rs = slice(hg * DM, (hg + 1) * DM)  # rows of W (output dims)
        in_maps.append(
            {
                "xq": np.ascontiguousarray(query[b]),
                "xk": np.ascontiguousarray(key[b]),
                "xv": np.ascontiguousarray(value[b]),
                "wqT": np.ascontiguousarray(Wqg[rs, :].T),
                "wkT": np.ascontiguousarray(Wkg[rs, :].T),
                "wvT": np.ascontiguousarray(Wvg[rs, :].T),
                "woT": np.ascontiguousarray(Wo[:, rs].T),
                "bq": np.ascontiguousarray(bq[rs]),
                "bk": np.ascontiguousarray(bk[rs]),
                "bv": np.ascontiguousarray(bv[rs]),
                "amask": amask_np[b],
                "ones_d": _ONES,
            }
        )

    runner = _get_runner()
    staged = runner.stage_inputs(in_maps)
    outs = runner.execute(staged)
    results = runner.to_numpy(outs)

    probs = np.empty((B, H, S, S), np.float32)
    context = np.zeros((B, S, D), np.float32)
    for core in range(8):
        b = core // 4
        hg = core % 4
        r = results[core]
        probs[b, hg * NH : (hg + 1) * NH] = r["probs4"]
        context[b] += r["ctxp"]
    context += bo[None, None, :]
    return context, probs
